# revision 14
# baseline (speedup 1.0000x reference)
"""Minibatch discrimination kernel for 8 trn2 NeuronCores — v3.

reference:
    M = (x @ T).reshape(B, K, D)                       # B=1024, K=50, D=5
    abs_diffs[i,k,j] = sum_d |M[i,k,d] - M[j,k,d]|
    feat[i,k] = sum_j exp(-abs_diffs[i,k,j])
    out = concat([x, feat], axis=1)                    # [1024, 562]

Sharding: kernels k split across 8 cores (K padded 50->56, 7 per core);
each core computes feat[:, its 7 k's] for ALL 1024 rows. The j-broadcast
of M^T rows is amortized over the 8 query i-tiles and done by DMA from a
DRAM staging buffer (SBUF sources would need partition alignment).

abs trick (|x| ops are not ISA-valid on DVE): |d| = d + 2*relu(-d), so
    L1[i,j] = (S[j] - S[i]) - 2*sum_d min(M[j,d]-M[i,d], 0),
    S[j] = sum_d M[j,d].
Per (k, i-tile) unit, exploiting symmetry of E = exp(-L1):
 - only j >= 128*it is computed (upper block-triangle, ~56% of work)
 - DVE: plane_d = min(bcast_d - mloc[:,d], 0) via ONE fused tensor_scalar
   (subtract -> min with 0), fp16 SBUF in/out, 4x perf mode
 - PE: matmuls accumulate into PSUM: S-row broadcast via +I, the 5 min
   planes via the stationary -2*I (scale and subtraction ride the
   weights); for wide tiles DVE/Pool pre-add plane pairs to offload PE
 - Scalar: E = Exp(-PSUM + S_local) via activation bias, fp16 out,
   accum_out = row-sum (diagonal + right-of-diagonal feat contribution)
 - PE: ones-vector matmuls column-sum E's off-diagonal 128-blocks =
   mirrored contribution to later i-tiles' feat
 - DVE: feat[:, (it,k)] = diag accum + mirrored accums
The S[j] term rides the broadcast as a 6th row per kernel slot; S_local
rides the M_local matmul as a 6th column (host passes [T | colsum(T)]
interleaved per slot) and enters exp via the bias operand.
"""

import sys

sys.path.insert(0, "/opt/trn_rl_repo")

from contextlib import ExitStack

import numpy as np

import concourse.bass as bass
import concourse.bacc as bacc
import concourse.tile as tile
from concourse import mybir
from concourse.bass_utils import run_bass_kernel_spmd

B, F = 1024, 512
K, D = 50, 5
NCORES = 8
KC = 7                # slots per core: 6 full kernels + 1 shared partial
KF = 6                # full (symmetric) kernel slots
SW = D + 1            # 6 staged rows/cols per kernel slot (5 m + 1 S)
CC = KC * SW          # 42 staged M^T rows / M_local cols per core
NT = 8                # query i-tiles of 128 rows
NP = 2                # this core's i-tiles of the shared partial kernel

f32 = mybir.dt.float32
f16 = mybir.dt.float16

# per-i-tile unit engine assignment:
#   'dve'  — min-planes on DVE (streams via -2*I with staged -S/2)
#   'pool' — min-planes on GpSimd (same -2*I stream scheme)
#   'act'  — |d| planes on ScalarE Abs (streams via +I, exp bias 0)
UNIT = ["dve", "dve", "act", "dve", "dve", "dve", "dve", "dve"]
# S-fold pre-add engine for dve/pool units ('pool'/'dve'/None = extra PE stream)
PRE = ["pool", "pool", None, "pool", "dve", "dve", None, None]

SUB = mybir.AluOpType.subtract
MIN = mybir.AluOpType.min
ADD = mybir.AluOpType.add


def _build_program():
    nc = bacc.Bacc("TRN2", target_bir_lowering=False)

    xT = nc.dram_tensor("xT", [F, B], f16, kind="ExternalInput").ap()
    xpart = nc.dram_tensor("xpart", [F, NP * 128], f16, kind="ExternalInput").ap()
    Tloc = nc.dram_tensor("Tloc", [F, CC], f16, kind="ExternalInput").ap()
    ident = nc.dram_tensor("ident", [128, 128], f16, kind="ExternalInput").ap()
    nident2 = nc.dram_tensor("nident2", [128, 128], f16, kind="ExternalInput").ap()
    ones = nc.dram_tensor("ones", [128, 1], f16, kind="ExternalInput").ap()
    feat = nc.dram_tensor("feat", [B, KF], f32, kind="ExternalOutput").ap()
    featp = nc.dram_tensor("featp", [NP * 128, 1], f32, kind="ExternalOutput").ap()
    # DRAM staging for broadcast rows: DMA-broadcast sources must be
    # partition-aligned in SBUF, but DRAM APs carry no such constraint.
    mt_dram = nc.dram_tensor("mt_dram", [CC, B], f16, kind="Internal").ap()

    with tile.TileContext(nc) as tc, ExitStack() as ctx:
        const_pool = ctx.enter_context(tc.tile_pool(name="const", bufs=1))
        mm_psum = ctx.enter_context(tc.tile_pool(name="mmps", bufs=2, space="PSUM"))
        l1_psum = ctx.enter_context(tc.tile_pool(name="l1ps", bufs=2, space="PSUM"))
        cs_psum = ctx.enter_context(tc.tile_pool(name="csps", bufs=2, space="PSUM"))
        bc_pool = ctx.enter_context(tc.tile_pool(name="bc", bufs=3))
        plane_pool = ctx.enter_context(tc.tile_pool(name="planes", bufs=3))
        e_pool = ctx.enter_context(tc.tile_pool(name="etile", bufs=3))
        small_pool = ctx.enter_context(tc.tile_pool(name="small", bufs=4))

        # ---- load inputs -------------------------------------------------
        xt_sb = []
        xp_sb = []
        tl_sb = []
        for fc in range(4):
            t = const_pool.tile([128, B], f16, tag=f"xt{fc}")
            nc.sync.dma_start(out=t[:], in_=xT[128 * fc : 128 * (fc + 1), :])
            xt_sb.append(t)
            t3 = const_pool.tile([128, NP * 128], f16, tag=f"xp{fc}")
            nc.sync.dma_start(out=t3[:], in_=xpart[128 * fc : 128 * (fc + 1), :])
            xp_sb.append(t3)
            t2 = const_pool.tile([128, CC], f16, tag=f"tl{fc}")
            nc.sync.dma_start(out=t2[:], in_=Tloc[128 * fc : 128 * (fc + 1), :])
            tl_sb.append(t2)
        id_sb = const_pool.tile([128, 128], f16, tag="ident")
        nc.sync.dma_start(out=id_sb[:], in_=ident[:, :])
        n2_sb = const_pool.tile([128, 128], f16, tag="nident2")
        nc.sync.dma_start(out=n2_sb[:], in_=nident2[:, :])
        ones_sb = const_pool.tile([128, 1], f16, tag="ones")
        nc.sync.dma_start(out=ones_sb[:], in_=ones[:, :])

        # PE may carry at most one sync wait per fused matmul (walrus
        # S3_LW limit); give PE a dummy matmul per input-DMA sem so real
        # matmuls below wait on at most one new sem.
        ps_dummy = mm_psum.tile([128, 512], f32, tag="mm", name="ps_dummy")
        for dt_tile in xt_sb + tl_sb + [id_sb, n2_sb]:
            nc.tensor.matmul(
                out=ps_dummy[0:32, 0:32],
                lhsT=dt_tile[0:32, 0:32],
                rhs=dt_tile[0:32, 0:32],
                start=True,
                stop=True,
                tile_position=(0, 0),
            )

        # ---- staged rows [CC, 1024] fp16: per slot 5 m-rows + 1 S-row ----
        # fast path: slot 0's rows first, so the k=0 broadcast (and the
        # whole DVE pipeline) starts ~10us earlier
        mt0_sb = const_pool.tile([128, B], f16, tag="mt0")
        for jh in range(2):
            ps = mm_psum.tile([128, 512], f32, tag="mm")
            for fc in range(4):
                nc.tensor.matmul(
                    out=ps[:SW, :],
                    lhsT=tl_sb[fc][:, :SW],
                    rhs=xt_sb[fc][:, 512 * jh : 512 * (jh + 1)],
                    start=(fc == 0),
                    stop=(fc == 3),
                )
            nc.scalar.copy(mt0_sb[:SW, 512 * jh : 512 * (jh + 1)], ps[:SW, :])
        nc.sync.dma_start(out=mt_dram[:SW, :], in_=mt0_sb[:SW, :])

        mt_sb = const_pool.tile([128, B], f16, tag="mt")
        for jh in range(2):
            ps = mm_psum.tile([128, 512], f32, tag="mm")
            for fc in range(4):
                nc.tensor.matmul(
                    out=ps[: CC - SW, :],
                    lhsT=tl_sb[fc][:, SW:CC],
                    rhs=xt_sb[fc][:, 512 * jh : 512 * (jh + 1)],
                    start=(fc == 0),
                    stop=(fc == 3),
                )
            nc.scalar.copy(
                mt_sb[: CC - SW, 512 * jh : 512 * (jh + 1)], ps[: CC - SW, :]
            )
        nc.sync.dma_start(out=mt_dram[SW:, :], in_=mt_sb[: CC - SW, :])

        # ---- M_local [128, (it, c)] f32 (c = slot-major, 6 per slot) -----
        mloc = const_pool.tile([128, NT * CC], f32, tag="mloc")
        for it in range(NT):
            ps = mm_psum.tile([128, 512], f32, tag="mm")
            for fc in range(4):
                nc.tensor.matmul(
                    out=ps[:, :CC],
                    lhsT=xt_sb[fc][:, 128 * it : 128 * (it + 1)],
                    rhs=tl_sb[fc][:, :CC],
                    start=(fc == 0),
                    stop=(fc == 3),
                )
            nc.scalar.copy(mloc[:, it * CC : (it + 1) * CC], ps[:, :CC])

        # exp bias: S_local = -2 * (the staged -S/2 columns of mloc)
        sbias = const_pool.tile([128, NT * KC], f32, tag="sbias")
        mls = mloc[:].rearrange("p (t k s) -> p t k s", k=KC, s=SW)
        nc.vector.tensor_scalar(
            sbias[:, :],
            mls[:, :, :, D : D + 1],
            -2.0,
            None,
            op0=mybir.AluOpType.mult,
        )


        # M_local + S bias for the shared partial kernel's 2 i-tiles
        mlocp = const_pool.tile([128, NP * SW], f32, tag="mlocp")
        for u in range(NP):
            ps = mm_psum.tile([128, 512], f32, tag="mm")
            for fc in range(4):
                nc.tensor.matmul(
                    out=ps[:, :SW],
                    lhsT=xp_sb[fc][:, 128 * u : 128 * (u + 1)],
                    rhs=tl_sb[fc][:, KF * SW : KC * SW],
                    start=(fc == 0),
                    stop=(fc == 3),
                )
            nc.scalar.copy(mlocp[:, u * SW : (u + 1) * SW], ps[:, :SW])
        sbp = const_pool.tile([128, NP], f32, tag="sbp")
        mlp = mlocp[:].rearrange("p (u s) -> p u s", s=SW)
        nc.vector.tensor_scalar(
            sbp[:, :], mlp[:, :, D : D + 1], -2.0, None, op0=mybir.AluOpType.mult
        )

        feat_sb = const_pool.tile([128, NT * KF], f32, tag="feat")
        fv = feat_sb[:].rearrange("p (t k) -> p t k", t=NT)

        # ---- shared partial kernel: NP full-width units, no symmetry -----
        bcp = bc_pool.tile([128, SW * B], f16, tag="bc")
        for d in range(SW):
            r = SW * KF + d
            src = mt_dram[r, :].partition_broadcast(128)
            eng = nc.gpsimd if d % 2 == 0 else nc.sync
            eng.dma_start(out=bcp[:, d * B : (d + 1) * B], in_=src)
        diagp = small_pool.tile([128, NP], f32, tag="diagp")
        for u in range(NP):
            # |d| on ScalarE (idle during startup); streams via +I, bias 0
            planes = plane_pool.tile([128, D * B], f16, tag="pl")
            for d in range(D):
                nc.scalar.activation(
                    planes[:, d * B : (d + 1) * B],
                    bcp[:, d * B : (d + 1) * B],
                    mybir.ActivationFunctionType.Abs,
                    bias=mlocp[:, u * SW + d : u * SW + d + 1],
                    scale=-1.0,
                )
            streams = [planes[:, d * B : (d + 1) * B] for d in range(D)]
            l1 = l1_psum.tile([128, B], f32, tag="l1")
            for c0 in range(0, B, 512):
                c1 = c0 + 512
                for si, srcp in enumerate(streams):
                    nc.tensor.matmul(
                        out=l1[:, c0:c1],
                        lhsT=id_sb[:, :],
                        rhs=srcp[:, c0:c1],
                        start=(si == 0),
                        stop=(si == len(streams) - 1),
                    )
            ep = e_pool.tile([128, B], f16, tag="e")
            nc.scalar.activation(
                ep[:, :],
                l1[:, :],
                mybir.ActivationFunctionType.Exp,
                bias=0.0,
                scale=-1.0,
                accum_out=diagp[:, u : u + 1],
            )
        fpv = featp[:, :].rearrange("(t p) one -> p t one", t=NP)
        nc.sync.dma_start(out=fpv, in_=diagp[:].rearrange("p (t o) -> p t o", o=1))

        # ---- main loop over this core's KF full kernels ------------------
        for k in range(KF):
            # broadcast slot k's 6 staged rows to all partitions, one DMA
            # per row so consumers start as soon as their row lands
            bc = bc_pool.tile([128, SW * B], f16, tag="bc")
            for d in range(SW):
                r = SW * k + d
                src = mt_dram[r, :].partition_broadcast(128)
                eng = nc.gpsimd if d % 2 == 0 else nc.sync
                eng.dma_start(out=bc[:, d * B : (d + 1) * B], in_=src)

            diag = small_pool.tile([128, NT], f32, tag="diag")
            # mirrored-contribution accumulators; two to halve the serial
            # add chain (even its -> mirA, odd -> mirB)
            mirA = small_pool.tile([128, NT], f32, tag="mirA")
            mirB = small_pool.tile([128, NT], f32, tag="mirB")
            nc.gpsimd.memset(mirA[:, :], 0.0)
            nc.gpsimd.memset(mirB[:, :], 0.0)

            for it in range(NT):
                off = 128 * it
                w = B - off
                mc = it * CC + SW * k
                planes = plane_pool.tile([128, D * B], f16, tag="pl")
                if UNIT[it] == "act":
                    # |d| directly on ScalarE; streams via +I, exp bias 0
                    for d in range(D):
                        nc.scalar.activation(
                            planes[:, d * w : (d + 1) * w],
                            bc[:, d * B + off : (d + 1) * B],
                            mybir.ActivationFunctionType.Abs,
                            bias=mloc[:, mc + d : mc + d + 1],
                            scale=-1.0,
                        )
                    streams = [planes[:, d * w : (d + 1) * w] for d in range(D)]
                    lhs = id_sb
                    ebias = 0.0
                else:
                    eng = nc.vector if UNIT[it] == "dve" else nc.gpsimd
                    for d in range(D):
                        eng.tensor_scalar(
                            planes[:, d * w : (d + 1) * w],
                            bc[:, d * B + off : (d + 1) * B],
                            mloc[:, mc + d : mc + d + 1],
                            0.0,
                            op0=SUB,
                            op1=MIN,
                        )
                    # all summands stream through the stationary -2*I: the
                    # min planes directly; the staged row holds -S/2 so it
                    # lands as +S[j]. A pre-add folds -S/2 into plane 3.
                    sv = bc[:, D * B + off : (D + 1) * B]
                    streams = [planes[:, d * w : (d + 1) * w] for d in range(D)]
                    if PRE[it] is not None:
                        p34 = small_pool.tile([128, B], f16, tag="p34")
                        peng = nc.vector if PRE[it] == "dve" else nc.gpsimd
                        peng.tensor_tensor(
                            out=p34[:, :w], in0=streams[3], in1=sv, op=ADD
                        )
                        streams = streams[:3] + [streams[4], p34[:, :w]]
                    else:
                        streams = streams + [sv]
                    lhs = n2_sb
                    ebias = sbias[:, it * KC + k : it * KC + k + 1]

                l1 = l1_psum.tile([128, B], f32, tag="l1")
                for c0 in range(0, w, 512):
                    c1 = min(c0 + 512, w)
                    for si, srcp in enumerate(streams):
                        nc.tensor.matmul(
                            out=l1[:, c0:c1],
                            lhsT=lhs[:, :],
                            rhs=srcp[:, c0:c1],
                            start=(si == 0),
                            stop=(si == len(streams) - 1),
                        )

                e = e_pool.tile([128, B], f16, tag="e")
                nc.scalar.activation(
                    e[:, :w],
                    l1[:, :w],
                    mybir.ActivationFunctionType.Exp,
                    bias=ebias,
                    scale=-1.0,
                    accum_out=diag[:, it : it + 1],
                )

                # column-sums of off-diagonal 128-blocks -> mirrored feat
                if it < NT - 1:
                    cs = cs_psum.tile([128, NT], f32, tag="cs")
                    for jt in range(it + 1, NT):
                        lo = 128 * (jt - it)
                        nc.tensor.matmul(
                            out=cs[:, jt : jt + 1],
                            lhsT=e[:, lo : lo + 128],
                            rhs=ones_sb[:, :],
                            start=True,
                            stop=True,
                        )
                    mir = mirA if it % 2 == 0 else mirB
                    nc.vector.tensor_tensor(
                        out=mir[:, it + 1 : NT],
                        in0=mir[:, it + 1 : NT],
                        in1=cs[:, it + 1 : NT],
                        op=ADD,
                    )

            dm = small_pool.tile([128, NT], f32, tag="dm")
            nc.vector.tensor_tensor(
                out=dm[:, :], in0=mirA[:, :], in1=diag[:, 0:NT], op=ADD
            )
            nc.vector.tensor_tensor(
                out=fv[:, 0:NT, k : k + 1],
                in0=dm[:, :],
                in1=mirB[:, :],
                op=ADD,
            )
            # stream this kernel's feat column out now; only the last k's
            # small DMA remains on the tail
            kv = feat[:, k : k + 1].rearrange("(t p) one -> p t one", t=NT)
            nc.sync.dma_start(out=kv, in_=fv[:, 0:NT, k : k + 1])

    nc.compile()
    return nc


_program_cache = {}


def _get_program():
    if "nc" not in _program_cache:
        _program_cache["nc"] = _build_program()
    return _program_cache["nc"]


def _make_consts():
    ident = np.zeros((128, 128), dtype=np.float16)
    np.fill_diagonal(ident, 1.0)
    nident2 = np.zeros((128, 128), dtype=np.float16)
    np.fill_diagonal(nident2, -2.0)
    ones = np.ones((128, 1), dtype=np.float16)
    return ident, nident2, ones


def make_in_maps(x, T):
    xT_full = np.ascontiguousarray(x.T.astype(np.float16))  # [512, 1024]
    Tk = T.reshape(F, K, D)
    ident, nident2, ones = _make_consts()
    in_maps = []
    for i in range(NCORES):
        # 6 full kernels + 1 shared partial kernel (48 for cores 0-3,
        # 49 for cores 4-7; each core covers 2 of its 8 i-tiles)
        slots = list(range(KF * i, KF * i + KF)) + [KF * NCORES + i // 4]
        Tl = np.zeros((F, KC, SW), dtype=np.float32)
        for s, g in enumerate(slots):
            Tl[:, s, :D] = Tk[:, g, :]
            Tl[:, s, D] = -0.5 * Tk[:, g, :].sum(axis=1)
        xp = xT_full[:, 256 * (i % 4) : 256 * (i % 4 + 1)]
        in_maps.append(
            {
                "xT": xT_full,
                "xpart": np.ascontiguousarray(xp),
                "Tloc": np.ascontiguousarray(
                    Tl.reshape(F, CC).astype(np.float16)
                ),
                "ident": ident,
                "nident2": nident2,
                "ones": ones,
            }
        )
    return in_maps


def kernel(x: np.ndarray, T: np.ndarray, _trace=False, _trace_kwargs=None):
    x = np.asarray(x, dtype=np.float32)
    T = np.asarray(T, dtype=np.float32)
    nc = _get_program()
    in_maps = make_in_maps(x, T)

    res = run_bass_kernel_spmd(
        nc,
        in_maps,
        core_ids=list(range(NCORES)),
        trace=_trace,
        **(_trace_kwargs or {}),
    )
    feats = np.zeros((B, K), dtype=np.float32)
    for c in range(NCORES):
        feats[:, KF * c : KF * (c + 1)] = res.results[c]["feat"]
        g = KF * NCORES + c // 4
        r0 = 256 * (c % 4)
        feats[r0 : r0 + 256, g] = res.results[c]["featp"][:, 0]
    out = np.concatenate([x, feats], axis=1)
    if _trace:
        return out, res
    return out


# revision 15
# speedup vs baseline: 1.0482x; 1.0482x over previous
"""Minibatch discrimination kernel for 8 trn2 NeuronCores — v3.

reference:
    M = (x @ T).reshape(B, K, D)                       # B=1024, K=50, D=5
    abs_diffs[i,k,j] = sum_d |M[i,k,d] - M[j,k,d]|
    feat[i,k] = sum_j exp(-abs_diffs[i,k,j])
    out = concat([x, feat], axis=1)                    # [1024, 562]

Sharding: kernels k split across 8 cores (K padded 50->56, 7 per core);
each core computes feat[:, its 7 k's] for ALL 1024 rows. The j-broadcast
of M^T rows is amortized over the 8 query i-tiles and done by DMA from a
DRAM staging buffer (SBUF sources would need partition alignment).

abs trick (|x| ops are not ISA-valid on DVE): |d| = d + 2*relu(-d), so
    L1[i,j] = (S[j] - S[i]) - 2*sum_d min(M[j,d]-M[i,d], 0),
    S[j] = sum_d M[j,d].
Per (k, i-tile) unit, exploiting symmetry of E = exp(-L1):
 - only j >= 128*it is computed (upper block-triangle, ~56% of work)
 - DVE: plane_d = min(bcast_d - mloc[:,d], 0) via ONE fused tensor_scalar
   (subtract -> min with 0), fp16 SBUF in/out, 4x perf mode
 - PE: matmuls accumulate into PSUM: S-row broadcast via +I, the 5 min
   planes via the stationary -2*I (scale and subtraction ride the
   weights); for wide tiles DVE/Pool pre-add plane pairs to offload PE
 - Scalar: E = Exp(-PSUM + S_local) via activation bias, fp16 out,
   accum_out = row-sum (diagonal + right-of-diagonal feat contribution)
 - PE: ones-vector matmuls column-sum E's off-diagonal 128-blocks =
   mirrored contribution to later i-tiles' feat
 - DVE: feat[:, (it,k)] = diag accum + mirrored accums
The S[j] term rides the broadcast as a 6th row per kernel slot; S_local
rides the M_local matmul as a 6th column (host passes [T | colsum(T)]
interleaved per slot) and enters exp via the bias operand.
"""

import sys

sys.path.insert(0, "/opt/trn_rl_repo")

from contextlib import ExitStack

import numpy as np

import concourse.bass as bass
import concourse.bacc as bacc
import concourse.tile as tile
from concourse import mybir
from concourse.bass_utils import run_bass_kernel_spmd

B, F = 1024, 512
K, D = 50, 5
NCORES = 8
KC = 7                # slots per core: 6 full kernels + 1 shared partial
KF = 6                # full (symmetric) kernel slots
SW = D + 1            # 6 staged rows/cols per kernel slot (5 m + 1 S)
CC = KC * SW          # 42 staged M^T rows / M_local cols per core
NT = 8                # query i-tiles of 128 rows
NP = 2                # this core's i-tiles of the shared partial kernel

f32 = mybir.dt.float32
f16 = mybir.dt.float16

# per-i-tile unit engine assignment:
#   'dve'  — min-planes on DVE (streams via -2*I with staged -S/2)
#   'pool' — min-planes on GpSimd (same -2*I stream scheme)
#   'act'  — |d| planes on ScalarE Abs (streams via +I, exp bias 0)
UNIT = ["dve", "dve", "act", "dve", "dve", "dve", "dve", "dve"]
# S-fold pre-add engine for dve/pool units ('pool'/'dve'/None = extra PE stream)
PRE = ["pool", "pool", None, "pool", "dve", "dve", None, None]

SUB = mybir.AluOpType.subtract
MIN = mybir.AluOpType.min
ADD = mybir.AluOpType.add


def _build_program():
    nc = bacc.Bacc("TRN2", target_bir_lowering=False)

    xT = nc.dram_tensor("xT", [F, B], f16, kind="ExternalInput").ap()
    xpart = nc.dram_tensor("xpart", [F, NP * 128], f16, kind="ExternalInput").ap()
    Tloc = nc.dram_tensor("Tloc", [F, CC], f16, kind="ExternalInput").ap()
    ident = nc.dram_tensor("ident", [128, 128], f16, kind="ExternalInput").ap()
    nident2 = nc.dram_tensor("nident2", [128, 128], f16, kind="ExternalInput").ap()
    ones = nc.dram_tensor("ones", [128, 1], f16, kind="ExternalInput").ap()
    feat = nc.dram_tensor("feat", [B, KF], f32, kind="ExternalOutput").ap()
    featp = nc.dram_tensor("featp", [NP * 128, 1], f32, kind="ExternalOutput").ap()
    # DRAM staging for broadcast rows: DMA-broadcast sources must be
    # partition-aligned in SBUF, but DRAM APs carry no such constraint.
    mt_dram = nc.dram_tensor("mt_dram", [CC, B], f16, kind="Internal").ap()

    with tile.TileContext(nc) as tc, ExitStack() as ctx:
        const_pool = ctx.enter_context(tc.tile_pool(name="const", bufs=1))
        mm_psum = ctx.enter_context(tc.tile_pool(name="mmps", bufs=2, space="PSUM"))
        l1_psum = ctx.enter_context(tc.tile_pool(name="l1ps", bufs=2, space="PSUM"))
        cs_psum = ctx.enter_context(tc.tile_pool(name="csps", bufs=2, space="PSUM"))
        bc_pool = ctx.enter_context(tc.tile_pool(name="bc", bufs=3))
        plane_pool = ctx.enter_context(tc.tile_pool(name="planes", bufs=3))
        e_pool = ctx.enter_context(tc.tile_pool(name="etile", bufs=3))
        small_pool = ctx.enter_context(tc.tile_pool(name="small", bufs=4))

        # ---- load inputs -------------------------------------------------
        xt_sb = []
        xp_sb = []
        tl_sb = []
        for fc in range(4):
            t = const_pool.tile([128, B], f16, tag=f"xt{fc}")
            nc.sync.dma_start(out=t[:], in_=xT[128 * fc : 128 * (fc + 1), :])
            xt_sb.append(t)
            t3 = const_pool.tile([128, NP * 128], f16, tag=f"xp{fc}")
            nc.sync.dma_start(out=t3[:], in_=xpart[128 * fc : 128 * (fc + 1), :])
            xp_sb.append(t3)
            t2 = const_pool.tile([128, CC], f16, tag=f"tl{fc}")
            nc.sync.dma_start(out=t2[:], in_=Tloc[128 * fc : 128 * (fc + 1), :])
            tl_sb.append(t2)
        id_sb = const_pool.tile([128, 128], f16, tag="ident")
        nc.sync.dma_start(out=id_sb[:], in_=ident[:, :])
        n2_sb = const_pool.tile([128, 128], f16, tag="nident2")
        nc.sync.dma_start(out=n2_sb[:], in_=nident2[:, :])
        ones_sb = const_pool.tile([128, 1], f16, tag="ones")
        nc.sync.dma_start(out=ones_sb[:], in_=ones[:, :])

        # PE may carry at most one sync wait per fused matmul (walrus
        # S3_LW limit); give PE a dummy matmul per input-DMA sem so real
        # matmuls below wait on at most one new sem.
        ps_dummy = mm_psum.tile([128, 512], f32, tag="mm", name="ps_dummy")
        for dt_tile in xt_sb + tl_sb + [id_sb, n2_sb]:
            nc.tensor.matmul(
                out=ps_dummy[0:32, 0:32],
                lhsT=dt_tile[0:32, 0:32],
                rhs=dt_tile[0:32, 0:32],
                start=True,
                stop=True,
                tile_position=(0, 0),
            )

        # ---- staged rows [CC, 1024] fp16: per slot 5 m-rows + 1 S-row ----
        # fast path: slot 0's rows first, so the k=0 broadcast (and the
        # whole DVE pipeline) starts ~10us earlier
        mt0_sb = const_pool.tile([128, B], f16, tag="mt0")
        for jh in range(2):
            ps = mm_psum.tile([128, 512], f32, tag="mm")
            for fc in range(4):
                nc.tensor.matmul(
                    out=ps[:SW, :],
                    lhsT=tl_sb[fc][:, :SW],
                    rhs=xt_sb[fc][:, 512 * jh : 512 * (jh + 1)],
                    start=(fc == 0),
                    stop=(fc == 3),
                )
            nc.scalar.copy(mt0_sb[:SW, 512 * jh : 512 * (jh + 1)], ps[:SW, :])
        nc.sync.dma_start(out=mt_dram[:SW, :], in_=mt0_sb[:SW, :])

        mt_sb = const_pool.tile([128, B], f16, tag="mt")
        for jh in range(2):
            ps = mm_psum.tile([128, 512], f32, tag="mm")
            for fc in range(4):
                nc.tensor.matmul(
                    out=ps[: CC - SW, :],
                    lhsT=tl_sb[fc][:, SW:CC],
                    rhs=xt_sb[fc][:, 512 * jh : 512 * (jh + 1)],
                    start=(fc == 0),
                    stop=(fc == 3),
                )
            nc.scalar.copy(
                mt_sb[: CC - SW, 512 * jh : 512 * (jh + 1)], ps[: CC - SW, :]
            )
        nc.sync.dma_start(out=mt_dram[SW:, :], in_=mt_sb[: CC - SW, :])

        # ---- M_local [128, (it, c)] f32 (c = slot-major, 6 per slot) -----
        mloc = const_pool.tile([128, NT * CC], f32, tag="mloc")
        for it in range(NT):
            ps = mm_psum.tile([128, 512], f32, tag="mm")
            for fc in range(4):
                nc.tensor.matmul(
                    out=ps[:, :CC],
                    lhsT=xt_sb[fc][:, 128 * it : 128 * (it + 1)],
                    rhs=tl_sb[fc][:, :CC],
                    start=(fc == 0),
                    stop=(fc == 3),
                )
            nc.scalar.copy(mloc[:, it * CC : (it + 1) * CC], ps[:, :CC])

        # exp bias: S_local = -2 * (the staged -S/2 columns of mloc)
        sbias = const_pool.tile([128, NT * KC], f32, tag="sbias")
        mls = mloc[:].rearrange("p (t k s) -> p t k s", k=KC, s=SW)
        nc.vector.tensor_scalar(
            sbias[:, :],
            mls[:, :, :, D : D + 1],
            -2.0,
            None,
            op0=mybir.AluOpType.mult,
        )


        # M_local + S bias for the shared partial kernel's 2 i-tiles
        mlocp = const_pool.tile([128, NP * SW], f32, tag="mlocp")
        for u in range(NP):
            ps = mm_psum.tile([128, 512], f32, tag="mm")
            for fc in range(4):
                nc.tensor.matmul(
                    out=ps[:, :SW],
                    lhsT=xp_sb[fc][:, 128 * u : 128 * (u + 1)],
                    rhs=tl_sb[fc][:, KF * SW : KC * SW],
                    start=(fc == 0),
                    stop=(fc == 3),
                )
            nc.scalar.copy(mlocp[:, u * SW : (u + 1) * SW], ps[:, :SW])
        sbp = const_pool.tile([128, NP], f32, tag="sbp")
        mlp = mlocp[:].rearrange("p (u s) -> p u s", s=SW)
        nc.vector.tensor_scalar(
            sbp[:, :], mlp[:, :, D : D + 1], -2.0, None, op0=mybir.AluOpType.mult
        )

        feat_sb = const_pool.tile([128, NT * KF], f32, tag="feat")
        fv = feat_sb[:].rearrange("p (t k) -> p t k", t=NT)

        # ---- shared partial kernel: NP full-width units, no symmetry -----
        bcp = bc_pool.tile([128, SW * B], f16, tag="bc")
        for d in range(SW):
            r = SW * KF + d
            src = mt_dram[r, :].partition_broadcast(128)
            eng = nc.gpsimd if d % 2 == 0 else nc.sync
            eng.dma_start(out=bcp[:, d * B : (d + 1) * B], in_=src)
        diagp = small_pool.tile([128, NP], f32, tag="diagp")
        for u in range(NP):
            planes = plane_pool.tile([128, D * B], f16, tag="pl")
            for d in range(D):
                nc.vector.tensor_scalar(
                    planes[:, d * B : (d + 1) * B],
                    bcp[:, d * B : (d + 1) * B],
                    mlocp[:, u * SW + d : u * SW + d + 1],
                    0.0,
                    op0=SUB,
                    op1=MIN,
                )
            p34 = small_pool.tile([128, B], f16, tag="p34")
            nc.gpsimd.tensor_tensor(
                out=p34[:, :],
                in0=planes[:, 3 * B : 4 * B],
                in1=bcp[:, D * B : (D + 1) * B],
                op=ADD,
            )
            streams = [
                planes[:, 0:B],
                planes[:, B : 2 * B],
                planes[:, 2 * B : 3 * B],
                planes[:, 4 * B : 5 * B],
                p34[:, :],
            ]
            l1 = l1_psum.tile([128, B], f32, tag="l1")
            for c0 in range(0, B, 512):
                c1 = c0 + 512
                for si, srcp in enumerate(streams):
                    nc.tensor.matmul(
                        out=l1[:, c0:c1],
                        lhsT=n2_sb[:, :],
                        rhs=srcp[:, c0:c1],
                        start=(si == 0),
                        stop=(si == len(streams) - 1),
                    )
            ep = e_pool.tile([128, B], f16, tag="e")
            nc.scalar.activation(
                ep[:, :],
                l1[:, :],
                mybir.ActivationFunctionType.Exp,
                bias=sbp[:, u : u + 1],
                scale=-1.0,
                accum_out=diagp[:, u : u + 1],
            )
        fpv = featp[:, :].rearrange("(t p) one -> p t one", t=NP)
        nc.sync.dma_start(out=fpv, in_=diagp[:].rearrange("p (t o) -> p t o", o=1))

        # ---- main loop over this core's KF full kernels ------------------
        for k in range(KF):
            # broadcast slot k's 6 staged rows to all partitions, one DMA
            # per row so consumers start as soon as their row lands
            bc = bc_pool.tile([128, SW * B], f16, tag="bc")
            for d in range(SW):
                r = SW * k + d
                src = mt_dram[r, :].partition_broadcast(128)
                eng = nc.gpsimd if d % 2 == 0 else nc.sync
                eng.dma_start(out=bc[:, d * B : (d + 1) * B], in_=src)

            diag = small_pool.tile([128, NT], f32, tag="diag")
            # mirrored-contribution accumulators; two to halve the serial
            # add chain (even its -> mirA, odd -> mirB)
            mirA = small_pool.tile([128, NT], f32, tag="mirA")
            mirB = small_pool.tile([128, NT], f32, tag="mirB")
            nc.gpsimd.memset(mirA[:, :], 0.0)
            nc.gpsimd.memset(mirB[:, :], 0.0)

            for it in range(NT):
                off = 128 * it
                w = B - off
                mc = it * CC + SW * k
                planes = plane_pool.tile([128, D * B], f16, tag="pl")
                if UNIT[it] == "act":
                    # |d| directly on ScalarE; streams via +I, exp bias 0
                    for d in range(D):
                        nc.scalar.activation(
                            planes[:, d * w : (d + 1) * w],
                            bc[:, d * B + off : (d + 1) * B],
                            mybir.ActivationFunctionType.Abs,
                            bias=mloc[:, mc + d : mc + d + 1],
                            scale=-1.0,
                        )
                    streams = [planes[:, d * w : (d + 1) * w] for d in range(D)]
                    lhs = id_sb
                    ebias = 0.0
                else:
                    eng = nc.vector if UNIT[it] == "dve" else nc.gpsimd
                    for d in range(D):
                        eng.tensor_scalar(
                            planes[:, d * w : (d + 1) * w],
                            bc[:, d * B + off : (d + 1) * B],
                            mloc[:, mc + d : mc + d + 1],
                            0.0,
                            op0=SUB,
                            op1=MIN,
                        )
                    # all summands stream through the stationary -2*I: the
                    # min planes directly; the staged row holds -S/2 so it
                    # lands as +S[j]. A pre-add folds -S/2 into plane 3.
                    sv = bc[:, D * B + off : (D + 1) * B]
                    streams = [planes[:, d * w : (d + 1) * w] for d in range(D)]
                    if PRE[it] is not None:
                        p34 = small_pool.tile([128, B], f16, tag="p34")
                        peng = nc.vector if PRE[it] == "dve" else nc.gpsimd
                        peng.tensor_tensor(
                            out=p34[:, :w], in0=streams[3], in1=sv, op=ADD
                        )
                        streams = streams[:3] + [streams[4], p34[:, :w]]
                    else:
                        streams = streams + [sv]
                    lhs = n2_sb
                    ebias = sbias[:, it * KC + k : it * KC + k + 1]

                l1 = l1_psum.tile([128, B], f32, tag="l1")
                for c0 in range(0, w, 512):
                    c1 = min(c0 + 512, w)
                    for si, srcp in enumerate(streams):
                        nc.tensor.matmul(
                            out=l1[:, c0:c1],
                            lhsT=lhs[:, :],
                            rhs=srcp[:, c0:c1],
                            start=(si == 0),
                            stop=(si == len(streams) - 1),
                        )

                e = e_pool.tile([128, B], f16, tag="e")
                nc.scalar.activation(
                    e[:, :w],
                    l1[:, :w],
                    mybir.ActivationFunctionType.Exp,
                    bias=ebias,
                    scale=-1.0,
                    accum_out=diag[:, it : it + 1],
                )

                # column-sums of off-diagonal 128-blocks -> mirrored feat
                if it < NT - 1:
                    cs = cs_psum.tile([128, NT], f32, tag="cs")
                    for jt in range(it + 1, NT):
                        lo = 128 * (jt - it)
                        nc.tensor.matmul(
                            out=cs[:, jt : jt + 1],
                            lhsT=e[:, lo : lo + 128],
                            rhs=ones_sb[:, :],
                            start=True,
                            stop=True,
                        )
                    mir = mirA if it % 2 == 0 else mirB
                    nc.vector.tensor_tensor(
                        out=mir[:, it + 1 : NT],
                        in0=mir[:, it + 1 : NT],
                        in1=cs[:, it + 1 : NT],
                        op=ADD,
                    )

            dm = small_pool.tile([128, NT], f32, tag="dm")
            nc.vector.tensor_tensor(
                out=dm[:, :], in0=mirA[:, :], in1=diag[:, 0:NT], op=ADD
            )
            nc.vector.tensor_tensor(
                out=fv[:, 0:NT, k : k + 1],
                in0=dm[:, :],
                in1=mirB[:, :],
                op=ADD,
            )
            # stream this kernel's feat column out now; only the last k's
            # small DMA remains on the tail
            kv = feat[:, k : k + 1].rearrange("(t p) one -> p t one", t=NT)
            nc.sync.dma_start(out=kv, in_=fv[:, 0:NT, k : k + 1])

    nc.compile()
    return nc


_program_cache = {}


def _get_program():
    if "nc" not in _program_cache:
        _program_cache["nc"] = _build_program()
    return _program_cache["nc"]


def _make_consts():
    ident = np.zeros((128, 128), dtype=np.float16)
    np.fill_diagonal(ident, 1.0)
    nident2 = np.zeros((128, 128), dtype=np.float16)
    np.fill_diagonal(nident2, -2.0)
    ones = np.ones((128, 1), dtype=np.float16)
    return ident, nident2, ones


def make_in_maps(x, T):
    xT_full = np.ascontiguousarray(x.T.astype(np.float16))  # [512, 1024]
    Tk = T.reshape(F, K, D)
    ident, nident2, ones = _make_consts()
    in_maps = []
    for i in range(NCORES):
        # 6 full kernels + 1 shared partial kernel (48 for cores 0-3,
        # 49 for cores 4-7; each core covers 2 of its 8 i-tiles)
        slots = list(range(KF * i, KF * i + KF)) + [KF * NCORES + i // 4]
        Tl = np.zeros((F, KC, SW), dtype=np.float32)
        for s, g in enumerate(slots):
            Tl[:, s, :D] = Tk[:, g, :]
            Tl[:, s, D] = -0.5 * Tk[:, g, :].sum(axis=1)
        xp = xT_full[:, 256 * (i % 4) : 256 * (i % 4 + 1)]
        in_maps.append(
            {
                "xT": xT_full,
                "xpart": np.ascontiguousarray(xp),
                "Tloc": np.ascontiguousarray(
                    Tl.reshape(F, CC).astype(np.float16)
                ),
                "ident": ident,
                "nident2": nident2,
                "ones": ones,
            }
        )
    return in_maps


def kernel(x: np.ndarray, T: np.ndarray, _trace=False, _trace_kwargs=None):
    x = np.asarray(x, dtype=np.float32)
    T = np.asarray(T, dtype=np.float32)
    nc = _get_program()
    in_maps = make_in_maps(x, T)

    res = run_bass_kernel_spmd(
        nc,
        in_maps,
        core_ids=list(range(NCORES)),
        trace=_trace,
        **(_trace_kwargs or {}),
    )
    feats = np.zeros((B, K), dtype=np.float32)
    for c in range(NCORES):
        feats[:, KF * c : KF * (c + 1)] = res.results[c]["feat"]
        g = KF * NCORES + c // 4
        r0 = 256 * (c % 4)
        feats[r0 : r0 + 256, g] = res.results[c]["featp"][:, 0]
    out = np.concatenate([x, feats], axis=1)
    if _trace:
        return out, res
    return out
